# revision 16
# baseline (speedup 1.0000x reference)
"""Trainium2 Bass kernel for nn_EngramAttention (causal MHA block).

Computes: qkv = x @ Wqkv + bqkv; causal 16-head attention; out @ Wout + bout.
Shapes: x [2, 2048, 1024], Wqkv [1024, 3072], Wout [1024, 1024].

Sharding (8 NeuronCores, tensor-parallel by heads):
  - core c owns heads {2c, 2c+1} (128 feature columns of each of Q/K/V).
  - Every core reads all tokens (x fed pre-transposed, feature-major, bf16).
  - Per batch b (= token half): QKV projection, causal attention in scoresT
    layout (keys on partitions; softmax denominator via an appended ones-row
    in the PV matmul).  Attention outputs are shipped UN-normalized together
    with the denominator row ([65, 256] blocks) through per-(batch, head)
    AllToAlls; each core ends with all 1024 features for its 256-token slice
    of each batch, normalizes post-A2A (one batched reciprocal + selection-
    matrix broadcast matmuls) and runs the output projection locally.
  - Emission weaves batch-1 QKV under batch-0 attention so the scalar engine
    (exp) starts early and the PE stream never starves (keeps HAM warm).

All matmuls run in bf16 (fp32 accumulation in PSUM).
"""

import os
import sys

for _p in ("/opt/trn_rl_repo", "/root/.axon_site/_ro/trn_rl_repo"):
    if os.path.isdir(_p) and _p not in sys.path:
        sys.path.insert(0, _p)

import ml_dtypes
import numpy as np

import concourse.bass as bass
import concourse.mybir as mybir
import concourse.tile as tile
from concourse.bass_utils import run_bass_kernel_spmd
from concourse.masks import make_identity
from concourse.vector_clock import ScopedClock

BF16 = mybir.dt.bfloat16
F32 = mybir.dt.float32
NPBF16 = ml_dtypes.bfloat16

NCORES = 8
D = 1024          # hidden
NTOK = 4096       # B*T
T = 2048
B = 2
FEAT = 128        # per-core head features (2 heads x 64)
TOKC = 256        # per-core per-batch token slice in the output projection
SCALE = 0.125     # 1/sqrt(64)

# module-level handles for optional tracing by test harnesses
TRACE = False
TRACE_KWARGS = {}
LAST_RESULT = None


class _SplitDrainTileContext(tile.TileContext):
    """TileContext whose tail drain splits semaphore waits one-per-instruction.

    The walrus build in this container rejects >N sync waits on a single
    Drain ("Too many sync wait commands"), so emit a chain of drains, each
    carrying a single wait, instead of one drain carrying all of them.
    """

    def _drain_and_barrier(self, tick_clock, wait_clock):
        nc = self.nc
        drain_inst = nc.sync.drain()
        wait_clock.add_sem_waits(
            drain_inst.ins, ScopedClock({None: tick_clock.global_clock})
        )
        si = drain_inst.ins.sync_info
        if si is not None and si.on_wait and len(si.on_wait) > 1:
            waits = list(si.on_wait)
            drain_inst.ins.sync_info = mybir.SyncInfo(
                on_wait=waits[:1], on_update=list(si.on_update or [])
            )
            for w in waits[1:]:
                d2 = nc.sync.drain()
                si2 = d2.ins.sync_info
                upd = list(si2.on_update or []) if si2 is not None else []
                d2.ins.sync_info = mybir.SyncInfo(on_wait=[w], on_update=upd)

        nc.all_engine_barrier()
        assert self.sems is not None
        popped = nc._tile_sem_poison_stack.pop()
        assert popped is self._sem_poison
        nc.clear_and_free_semaphores(list(self.sems.allocated().values()))
        nc.all_engine_barrier()


def _split_excess_waits(nc, aux, max_waits=1):
    """Walrus in this container rejects instructions carrying more than a
    couple of semaphore waits ("Too many sync wait commands").  Move excess
    waits onto EventSemaphore carrier instructions inserted just before the
    offending instruction on the same engine (same-engine FIFO order makes
    this semantically identical).

    DMA instructions execute on the DMA-queue processors, asynchronously
    from the issuing engine's stream, so an engine-side carrier alone would
    NOT order them (CoreSim race detector confirms).  For those, the carrier
    chain additionally increments an auxiliary semaphore and the DMA itself
    waits on it — the DMA then carries exactly one wait."""
    n = 0
    aux_count = 0
    dma_ops = ("DMACopy", "DMATranspose", "TriggeredCopy")

    def _carrier(engine, wait_grp):
        nonlocal n
        ev = mybir.InstEventSemaphore(
            name=f"wsplit-{n}",
            engine=engine,
            ins=[],
            outs=[],
            sync_info=mybir.SyncInfo(on_wait=list(wait_grp), on_update=[]),
        )
        n += 1
        nc.register_instruction(ev, overwrite=True)
        return ev

    for fn in nc.m.functions:
        for blk in fn.blocks:
            out = []
            for ins in blk.instructions:
                si = ins.sync_info
                waits = list(si.on_wait) if (si is not None and si.on_wait) else []
                if len(waits) > max_waits:
                    if ins.opcode in dma_ops:
                        for w in waits:
                            out.append(_carrier(ins.engine, [w]))
                        bass.BassInstruction(out[-1]).then_inc(aux, 1)
                        aux_count += 1
                        ins.sync_info = mybir.SyncInfo(
                            on_wait=[], on_update=list(si.on_update or [])
                        )
                        bass.BassInstruction(ins).wait_op(
                            aux, aux_count, "sem-ge"
                        )
                    else:
                        extra, keep = waits[:-max_waits], waits[-max_waits:]
                        for i in range(0, len(extra), max_waits):
                            out.append(_carrier(ins.engine, extra[i : i + max_waits]))
                        ins.sync_info = mybir.SyncInfo(
                            on_wait=keep, on_update=list(si.on_update or [])
                        )
                out.append(ins)
            blk.instructions = out
    if aux_count:
        # sems persist across NEFF executions; reset so a re-run starts at 0
        nc.gpsimd.sem_clear(range(aux.num, aux.num + 1))
    return n


def _weave(*streams, weights=None):
    """Emit thunks from several streams interleaved by fractional progress.
    A stream with weight w emits w times faster than a weight-1 stream, so
    higher-weight streams are front-loaded within the block."""
    pairs = [
        (list(s), (weights[i] if weights else 1.0))
        for i, s in enumerate(streams)
        if s
    ]
    idx = [0] * len(pairs)
    total = sum(len(s) for s, _ in pairs)
    for _ in range(total):
        k = min(
            range(len(pairs)),
            key=lambda i: (idx[i] / (len(pairs[i][0]) * pairs[i][1]), i)
            if idx[i] < len(pairs[i][0])
            else (9.0, i),
        )
        pairs[k][0][idx[k]]()
        idx[k] += 1


def _build_nc():
    nc = bass.Bass("TRN2", num_devices=NCORES)

    xT = nc.dram_tensor("xT", [D, NTOK], BF16, kind="ExternalInput")
    wq = nc.dram_tensor("wq", [D, FEAT], BF16, kind="ExternalInput")
    wk = nc.dram_tensor("wk", [D, FEAT], BF16, kind="ExternalInput")
    wv = nc.dram_tensor("wv", [D, FEAT], BF16, kind="ExternalInput")
    bq = nc.dram_tensor("bq", [FEAT, 1], F32, kind="ExternalInput")
    bk = nc.dram_tensor("bk", [FEAT, 1], F32, kind="ExternalInput")
    bv = nc.dram_tensor("bv", [FEAT, 1], F32, kind="ExternalInput")
    wout = nc.dram_tensor("wout", [D, D], BF16, kind="ExternalInput")
    boutb = nc.dram_tensor("boutb", [1, D], BF16, kind="ExternalInput")
    maskg = nc.dram_tensor("maskg", [128, 896], BF16, kind="ExternalInput")
    selg = nc.dram_tensor("selg", [64, 8 * 128], BF16, kind="ExternalInput")
    y = nc.dram_tensor("y", [2 * TOKC, D], F32, kind="ExternalOutput")

    # auxiliary semaphore for the DMA-wait splitting pass; allocated (and
    # cleared) before the TileContext so Tile never recycles its ID
    aux_sem = nc.alloc_semaphore("wsplit_aux")
    nc.gpsimd.sem_clear(range(aux_sem.num, aux_sem.num + 1))

    with _SplitDrainTileContext(nc) as tc:
        with (
            tc.tile_pool(name="const", bufs=1) as cp,
            tc.tile_pool(name="work", bufs=2) as wp,
            tc.tile_pool(name="vtp", bufs=2) as vp,
            tc.tile_pool(name="stage", bufs=2) as sp2,
            tc.tile_pool(name="psA", bufs=3, space="PSUM") as psA,
            tc.tile_pool(name="psB", bufs=2, space="PSUM") as psB,
            tc.tile_pool(name="dram", bufs=1, space="DRAM") as dp,
        ):
            # ---- persistent SBUF tensors ----
            xt_sb = cp.tile([128, 8 * NTOK], BF16, name="xt_sb")     # 64 KB/part
            wq_sb = cp.tile([128, 8 * FEAT], BF16, name="wq_sb")
            wk_sb = cp.tile([128, 8 * FEAT], BF16, name="wk_sb")
            wv_sb = cp.tile([128, 8 * FEAT], BF16, name="wv_sb")
            bq_sb = cp.tile([FEAT, 1], F32, name="bq_sb")
            bk_sb = cp.tile([FEAT, 1], F32, name="bk_sb")
            bv_sb = cp.tile([FEAT, 1], F32, name="bv_sb")
            bout_sb = cp.tile([1, D], BF16, name="bout_sb")
            mask_sb = cp.tile([128, 896], BF16, name="mask_sb")
            ident_sb = cp.tile([128, 128], BF16, name="ident_sb")
            ones1_sb = cp.tile([1, 128], BF16, name="ones1_sb")
            sel_sb = cp.tile([64, 8 * 128], BF16, name="sel_sb")
            dummy_sb = cp.tile([128, 128], BF16, name="dummy_sb")
            qT_sb = cp.tile([128, NTOK], BF16, name="qT_sb")
            kT_sb = cp.tile([128, NTOK], BF16, name="kT_sb")
            vtok_sb = cp.tile([128, 32 * 130], BF16, name="vtok_sb")
            wout_sb = cp.tile([128, 8 * D], BF16, name="wout_sb")
            ag_sb = [
                cp.tile([128, 8 * TOKC], BF16, name=f"ag_sb{b}") for b in range(2)
            ]
            denb_sb = [
                cp.tile([64, TOKC], BF16, name=f"denb_sb{b}") for b in range(2)
            ]
            denf_sb = [
                cp.tile([64, TOKC], F32, name=f"denf_sb{b}") for b in range(2)
            ]
            denr_sb = [
                cp.tile([64, TOKC], F32, name=f"denr_sb{b}") for b in range(2)
            ]
            denrb_sb = [
                cp.tile([64, TOKC], BF16, name=f"denrb_sb{b}") for b in range(2)
            ]

            # ---- input DMAs ----
            # xT (batch-0 half first) on the sync HWDGE ring; weights/bias/
            # mask go via the gpsimd SWDGE ring so they don't delay xT.
            for bb in range(2):
                for kt in range(8):
                    nc.sync.dma_start(
                        xt_sb[:, kt * NTOK + bb * T : kt * NTOK + (bb + 1) * T],
                        xT[kt * 128 : (kt + 1) * 128, bb * T : (bb + 1) * T],
                    )
            for w_sb, wdr in ((wq_sb, wq), (wk_sb, wk), (wv_sb, wv)):
                for kt in range(8):
                    nc.gpsimd.dma_start(
                        w_sb[:, kt * FEAT : (kt + 1) * FEAT],
                        wdr[kt * 128 : (kt + 1) * 128, :],
                    )
            nc.gpsimd.dma_start(bq_sb[:], bq[:])
            nc.gpsimd.dma_start(bk_sb[:], bk[:])
            nc.gpsimd.dma_start(bv_sb[:], bv[:])
            nc.gpsimd.dma_start(bout_sb[:], boutb[:])
            nc.gpsimd.dma_start(mask_sb[:], maskg[:])
            for kt in range(8):
                nc.gpsimd.dma_start(
                    wout_sb[:, kt * D : (kt + 1) * D],
                    wout[kt * 128 : (kt + 1) * 128, :],
                )

            make_identity(nc, ident_sb[:])
            nc.vector.memset(ones1_sb[:], 1.0)
            nc.vector.memset(dummy_sb[:], 1.0)
            vt_view = vtok_sb[:].rearrange("p (g c) -> p g c", c=130)
            nc.vector.memset(vt_view[:, :, 64], 1.0)
            nc.vector.memset(vt_view[:, :, 129], 1.0)
            # selection matrix (host-built): sel[32h+r, r*128 + 64h + i] = 1
            # (i < 64), so sel[:, r*128:(r+1)*128].T @ denrb broadcasts den
            # row 32h+r onto output partitions [64h, 64h+64) for block r.
            # (h=1 rows live at partition 32 — engine APs need 32-aligned
            # partition starts.)  Unused denrb rows must be zero so the
            # broadcast matmul never multiplies 0 * garbage.
            nc.gpsimd.dma_start(sel_sb[:], selg[:])
            for bb in range(2):
                nc.vector.memset(denrb_sb[bb][:], 0.0)

            _dummy_phase = [0]

            def dummy_ops(nmm, pool=None, tag="mm2"):
                """Full-utilization keep-warm matmuls with no data deps.
                Operands come from wout (random data) so the PE datapath
                actually toggles — constant operands don't register as
                activity for the clock-gate monitor."""
                ops = []
                group = 8
                for i in range(0, nmm, group):
                    def op(n=min(group, nmm - i), pool=pool or psA, tag=tag):
                        ps_d = pool.tile([128, 128], F32, tag=tag, name="ps_d")
                        for _ in range(n):
                            j = _dummy_phase[0] = (_dummy_phase[0] + 5) % 9
                            nc.tensor.matmul(
                                ps_d[:],
                                xt_sb[:, j * 128 : (j + 1) * 128],
                                xt_sb[:, (j + 7) * 128 : (j + 8) * 128],
                                start=True, stop=True,
                            )
                    ops.append(op)
                return ops

            # ---- QKV projection: dstT[f, tok] = W.T @ x.T (+ bias) ----
            def qkv_ops(which, bb, ts=(0, 1), box=None):
                w_sb, b_sb, dst = {
                    "q": (wq_sb, bq_sb, qT_sb),
                    "k": (wk_sb, bk_sb, kT_sb),
                    "v": (wv_sb, bv_sb, None),
                }[which]
                ops = []
                if box is None:
                    box = {}
                for t in ts:
                    def alloc(t=t):
                        if which == "v" and "vt" not in box:
                            box["vt"] = vp.tile([128, T], BF16, name=f"vT{bb}")
                        box["ps"] = psA.tile(
                            [128, 1024], F32, tag="mm2", name=f"ps_{which}{bb}{t}"
                        )
                    ops.append(alloc)
                    for kt in range(8):
                        def op(t=t, kt=kt):
                            ps = box["ps"]
                            for c in range(2):
                                base = bb * T + t * 1024 + c * 512
                                nc.tensor.matmul(
                                    ps[:, c * 512 : (c + 1) * 512],
                                    w_sb[:, kt * FEAT : (kt + 1) * FEAT],
                                    xt_sb[:, kt * NTOK + base : kt * NTOK + base + 512],
                                    start=(kt == 0),
                                    stop=(kt == 7),
                                )
                        ops.append(op)

                    def bias(t=t):
                        ps = box["ps"]
                        d = box["vt"] if which == "v" else dst
                        off = t * 1024 if which == "v" else bb * T + t * 1024
                        nc.vector.tensor_scalar_add(
                            d[:, off : off + 1024], ps[:], b_sb[:]
                        )
                    ops.append(bias)
                return ops, box

            # ---- v to token-major (PE transposes), with ones columns ----
            def vpost_ops(bb, vbox):
                ops = []
                for gl in range(16):
                    def op(gl=gl):
                        g = bb * 16 + gl
                        ps_t = psA.tile([128, 128], BF16, tag="mm2", name="ps_t")
                        nc.tensor.transpose(
                            ps_t[:], vbox["vt"][:, gl * 128 : (gl + 1) * 128],
                            ident_sb[:],
                        )
                        nc.vector.tensor_copy(
                            vtok_sb[:, g * 130 : g * 130 + 64], ps_t[:, 0:64]
                        )
                        nc.vector.tensor_copy(
                            vtok_sb[:, g * 130 + 65 : g * 130 + 129],
                            ps_t[:, 64:128],
                        )
                    ops.append(op)
                return ops

            # ---- attention stages ----
            # stage s = (h, b, j): q-chunk j (512 tokens of batch b), head
            # half h.  scoresT layout: [128 keys, 512 q] blocks, exp on ACT,
            # diagonal blocks masked, PV accumulates [65, 512] (row 64 =
            # softmax denominator via the vtok ones column).  The result is
            # shipped UN-normalized + denominator through the A2A.
            pt_tiles = {}
            a2a_in = {}
            a2a_out = {}
            for bb in range(2):
                for h in range(2):
                    a2a_in[bb, h] = dp.tile(
                        [8, 65, TOKC], BF16, name=f"a2a_in{bb}{h}"
                    )
                    a2a_out[bb, h] = dp.tile(
                        [8, 65, TOKC], BF16, name=f"a2a_out{bb}{h}"
                    )

            def scores_ops(s):
                h, bb, j = s
                nk = 4 * (j + 1)
                pt = wp.tile(
                    [128, nk * 512], BF16, tag="pt", name=f"pt_{h}_{bb}_{j}"
                )
                pt_tiles[s] = pt
                pb, cb = 64 * h, bb * T
                ops = []
                for kp in range(nk // 2):
                    def op(kp=kp, pt=pt, pb=pb, cb=cb, j=j):
                        ps2 = psA.tile([128, 1024], F32, tag="mm2", name="ps2")
                        for c in range(2):
                            kk = 2 * kp + c
                            nc.tensor.matmul(
                                ps2[:, c * 512 : (c + 1) * 512],
                                kT_sb[
                                    pb : pb + 64,
                                    cb + kk * 128 : cb + (kk + 1) * 128,
                                ],
                                qT_sb[
                                    pb : pb + 64,
                                    cb + j * 512 : cb + (j + 1) * 512,
                                ],
                                start=True,
                                stop=True,
                            )
                        nc.scalar.activation(
                            pt[:, (2 * kp) * 512 : (2 * kp + 2) * 512],
                            ps2[:],
                            mybir.ActivationFunctionType.Exp,
                            scale=SCALE,
                        )
                        for c in range(2):
                            kk = 2 * kp + c
                            if kk >= 4 * j:
                                i = kk - 4 * j
                                nc.vector.tensor_tensor(
                                    pt[:, kk * 512 : (kk + 1) * 512],
                                    pt[:, kk * 512 : (kk + 1) * 512],
                                    mask_sb[:, 384 - 128 * i : 896 - 128 * i],
                                    mybir.AluOpType.mult,
                                )
                    ops.append(op)
                return ops

            def pv_ops(s):
                h, bb, j = s
                nk = 4 * (j + 1)
                pt = pt_tiles.pop(s)
                ps_box = {}
                ops = []
                for kk in range(nk):
                    def op(kk=kk, pt=pt, h=h, bb=bb, nk=nk):
                        if kk == 0:
                            ps_box["o"] = psB.tile(
                                [65, 512], F32, tag="pv", name="ps_o"
                            )
                        g = bb * 16 + kk
                        nc.tensor.matmul(
                            ps_box["o"][:],
                            vtok_sb[:, g * 130 + 65 * h : g * 130 + 65 * h + 65],
                            pt[:, kk * 512 : (kk + 1) * 512],
                            start=(kk == 0),
                            stop=(kk == nk - 1),
                        )
                    ops.append(op)

                def ship(h=h, bb=bb, j=j):
                    ps_o = ps_box["o"]
                    av = sp2.tile([65, 512], BF16, tag="av", name="av")
                    nc.vector.tensor_copy(av[:], ps_o[:])
                    for t in range(2):
                        nc.sync.dma_start(
                            a2a_in[bb, h][2 * j + t],
                            av[:, t * TOKC : (t + 1) * TOKC],
                        )
                ops.append(ship)
                return ops

            def emit_collective(bb, h):
                nc.gpsimd.collective_compute(
                    "AllToAll",
                    mybir.AluOpType.bypass,
                    replica_groups=[list(range(NCORES))],
                    ins=[a2a_in[bb, h][:]],
                    outs=[a2a_out[bb, h][:]],
                )
                for r in range(8):
                    nc.gpsimd.dma_start(
                        ag_sb[bb][
                            64 * h : 64 * h + 64, r * TOKC : (r + 1) * TOKC
                        ],
                        a2a_out[bb, h][r, 0:64, :],
                    )
                nc.gpsimd.dma_start(
                    denb_sb[bb][32 * h : 32 * h + 8, :],
                    a2a_out[bb, h][:, 64, :],
                )

            # ---- post-A2A normalization + output projection ----
            def recip_ops(bb, h):
                def op():
                    sl = slice(32 * h, 32 * h + 8)
                    nc.vector.tensor_copy(denf_sb[bb][sl, :], denb_sb[bb][sl, :])
                    nc.vector.reciprocal(denr_sb[bb][sl, :], denf_sb[bb][sl, :])
                    nc.vector.tensor_copy(denrb_sb[bb][sl, :], denr_sb[bb][sl, :])
                return [op]

            def norm_ops(bb):
                ops = []
                for r in range(8):
                    def op(r=r):
                        ps_bc = psA.tile(
                            [128, TOKC], F32, tag="mm2", name="ps_bc"
                        )
                        nc.tensor.matmul(
                            ps_bc[:],
                            sel_sb[:, r * 128 : (r + 1) * 128],
                            denrb_sb[bb][:, :],
                            start=True,
                            stop=True,
                        )
                        nc.vector.tensor_tensor(
                            ag_sb[bb][:, r * TOKC : (r + 1) * TOKC],
                            ag_sb[bb][:, r * TOKC : (r + 1) * TOKC],
                            ps_bc[:],
                            mybir.AluOpType.mult,
                        )
                    ops.append(op)
                return ops

            def outproj_ops(bb):
                ops = []
                for m in range(2):
                    box = {}
                    def mm0(m=m, box=box):
                        box["ps"] = psA.tile(
                            [128, 1024], F32, tag="mm2", name=f"ps_y{bb}{m}"
                        )
                        for n2 in range(2):
                            nc.tensor.matmul(
                                box["ps"][:, n2 * 512 : (n2 + 1) * 512],
                                ones1_sb[0:1, 0:128],
                                bout_sb[:, n2 * 512 : (n2 + 1) * 512],
                                start=True,
                                stop=False,
                            )
                    ops.append(mm0)
                    for kt in range(8):
                        def op(m=m, kt=kt, box=box):
                            for n2 in range(2):
                                nc.tensor.matmul(
                                    box["ps"][:, n2 * 512 : (n2 + 1) * 512],
                                    ag_sb[bb][
                                        :,
                                        kt * TOKC + m * 128 :
                                        kt * TOKC + (m + 1) * 128,
                                    ],
                                    wout_sb[
                                        :,
                                        kt * D + n2 * 512 :
                                        kt * D + (n2 + 1) * 512,
                                    ],
                                    start=False,
                                    stop=(kt == 7),
                                )
                        ops.append(op)

                    def fin(m=m, box=box):
                        y_sb = sp2.tile([128, D], F32, tag="ysb", name="y_sb")
                        nc.vector.tensor_copy(y_sb[:], box["ps"][:])
                        nc.sync.dma_start(
                            y[bb * 256 + m * 128 : bb * 256 + (m + 1) * 128, :],
                            y_sb[:],
                        )
                    ops.append(fin)
                return ops

            def norm_half_ops(bb, h):
                """Normalize only the 64 feature rows of head-half h."""
                ops = []
                for r in range(8):
                    def op(r=r):
                        ps_bc = psB.tile(
                            [128, TOKC], F32, tag="pv", name="ps_bch"
                        )
                        nc.tensor.matmul(
                            ps_bc[64 * h : 64 * h + 64, :],
                            sel_sb[:, r * 128 + 64 * h : r * 128 + 64 * h + 64],
                            denrb_sb[bb][:, :],
                            start=True,
                            stop=True,
                        )
                        nc.vector.tensor_tensor(
                            ag_sb[bb][64 * h : 64 * h + 64,
                                      r * TOKC : (r + 1) * TOKC],
                            ag_sb[bb][64 * h : 64 * h + 64,
                                      r * TOKC : (r + 1) * TOKC],
                            ps_bc[64 * h : 64 * h + 64, :],
                            mybir.AluOpType.mult,
                        )
                    ops.append(op)
                return ops

            def outproj_partial_ops(bb, h, boxes, first, last):
                """Half-contraction output projection over feature rows of
                head-half h only (rows 64h:64h+64 of every kt block).  The
                PSUM accumulation group stays open between the two halves so
                the h=0 half can run while the last A2A is in flight."""
                ops = []
                for m in range(2):
                    box = boxes[m]
                    if first:
                        def mm0(m=m, box=box):
                            box["ps"] = psA.tile(
                                [128, 1024], F32, tag="mm2", name=f"ps_yp{m}"
                            )
                            for n2 in range(2):
                                nc.tensor.matmul(
                                    box["ps"][:, n2 * 512 : (n2 + 1) * 512],
                                    ones1_sb[0:1, 0:128],
                                    bout_sb[:, n2 * 512 : (n2 + 1) * 512],
                                    start=True,
                                    stop=False,
                                )
                        ops.append(mm0)
                    for kt in range(8):
                        def op(m=m, kt=kt, box=box):
                            for n2 in range(2):
                                nc.tensor.matmul(
                                    box["ps"][:, n2 * 512 : (n2 + 1) * 512],
                                    ag_sb[bb][
                                        64 * h : 64 * h + 64,
                                        kt * TOKC + m * 128 :
                                        kt * TOKC + (m + 1) * 128,
                                    ],
                                    wout_sb[
                                        64 * h : 64 * h + 64,
                                        kt * D + n2 * 512 :
                                        kt * D + (n2 + 1) * 512,
                                    ],
                                    start=False,
                                    stop=(last and kt == 7),
                                )
                        ops.append(op)
                    if last:
                        def fin(m=m, box=box):
                            y_sb = sp2.tile(
                                [128, D], F32, tag="ysb", name="y_sb"
                            )
                            nc.vector.tensor_copy(y_sb[:], box["ps"][:])
                            nc.sync.dma_start(
                                y[bb * 256 + m * 128 :
                                  bb * 256 + (m + 1) * 128, :],
                                y_sb[:],
                            )
                        ops.append(fin)
                return ops

            # ================= emission schedule =================
            # Phase A: only the first token-half of batch-0 K and Q — the
            # j=0,1 attention stages need nothing else, so the scalar engine
            # (exp, the long pole) starts ~25us earlier.  Warm-up dummies
            # keep the PE array active through the DMA-gated start.
            kops_t0, _ = qkv_ops("k", 0, ts=(0,))
            _weave(kops_t0, dummy_ops(40))
            for op in qkv_ops("q", 0, ts=(0,))[0]:
                op()

            # Rest of batch-0 QKV (k/q t1, v both halves) woven into the
            # first b0 stage; the v transposes are emitted as a block after
            # it so no b0 PV op can queue ahead of the transpose it needs.
            vbox0 = {}
            rest0 = (
                qkv_ops("k", 0, ts=(1,))[0]
                + qkv_ops("q", 0, ts=(1,))[0]
                + qkv_ops("v", 0, box=vbox0)[0]
            )
            rest0_chunks = [rest0] + [[]] * 7

            kops1, _ = qkv_ops("k", 1)
            qops1, _ = qkv_ops("q", 1)
            vops1, vbox1 = qkv_ops("v", 1)
            qkv1 = kops1 + qops1 + vops1 + vpost_ops(1, vbox1)
            nq = len(qkv1)
            # spread batch-1 QKV over b0 stages 1..7 (full-util matmuls keep
            # the PE activity monitor from throttling the clock)
            qkv1_chunks = [[]] + [
                qkv1[nq * i // 7 : nq * (i + 1) // 7] for i in range(7)
            ]

            # batch-0 post work woven into batch-1 stages 3..7; batch-1
            # h0 stages get dummy full-util matmuls as activity filler.
            post0 = (
                recip_ops(0, 0)
                + recip_ops(0, 1)
                + norm_ops(0)
                + outproj_ops(0)
            )
            np0 = len(post0)
            post0_chunks = [
                dummy_ops(24, ), dummy_ops(32), dummy_ops(40)
            ] + [
                post0[np0 * i // 5 : np0 * (i + 1) // 5] for i in range(5)
            ]

            stages = [(h, bb, j) for bb in range(2) for h in range(2)
                      for j in range(4)]
            prev = None
            for si, s in enumerate(stages):
                h, bb, j = s
                a = scores_ops(s)
                b = pv_ops(prev) if prev is not None else []
                if bb == 0:
                    _weave(a, b, rest0_chunks[si], qkv1_chunks[si],
                           weights=(3.0, 2.0, 1.0, 1.0))
                else:
                    _weave(a, b, post0_chunks[si - 8],
                           weights=(3.0, 2.0, 1.0))
                if si == 0:
                    for op in vpost_ops(0, vbox0):
                        op()
                if prev is not None and prev[2] == 3:
                    emit_collective(prev[1], prev[0])
                prev = s
            for op in pv_ops(prev):
                op()
            emit_collective(1, 1)

            # tail: h0 half of the batch-1 normalization + output projection
            # runs while the last A2A (which carries the h1 half) flies; the
            # PSUM accumulation groups stay open across the boundary.
            yboxes = [{}, {}]
            for op in recip_ops(1, 0) + norm_half_ops(1, 0):
                op()
            for op in outproj_partial_ops(1, 0, yboxes, first=True,
                                          last=False):
                op()
            for op in dummy_ops(48, pool=psB, tag="pv"):
                op()
            for op in recip_ops(1, 1) + norm_half_ops(1, 1):
                op()
            for op in outproj_partial_ops(1, 1, yboxes, first=False,
                                          last=True):
                op()

    _split_excess_waits(nc, aux_sem)
    return nc


_NC_CACHE = None


def _get_nc():
    global _NC_CACHE
    if _NC_CACHE is None:
        _NC_CACHE = _build_nc()
    return _NC_CACHE


def kernel(x, Wqkv, bqkv, Wout, bout):
    global LAST_RESULT
    x = np.asarray(x, dtype=np.float32)
    Wqkv = np.asarray(Wqkv, dtype=np.float32)
    bqkv = np.asarray(bqkv, dtype=np.float32)
    Wout = np.asarray(Wout, dtype=np.float32)
    bout = np.asarray(bout, dtype=np.float32)

    Bx, Tx, Dx = x.shape
    assert (Bx, Tx, Dx) == (B, T, D)

    xT = np.ascontiguousarray(x.reshape(NTOK, D).T).astype(NPBF16)
    wq_full = Wqkv[:, 0:D]
    wk_full = Wqkv[:, D : 2 * D]
    wv_full = Wqkv[:, 2 * D : 3 * D]
    bq_full = bqkv[0:D]
    bk_full = bqkv[D : 2 * D]
    bv_full = bqkv[2 * D : 3 * D]

    wout_b = np.ascontiguousarray(Wout).astype(NPBF16)
    boutb = np.ascontiguousarray(bout.reshape(1, D)).astype(NPBF16)
    maskg = (
        np.arange(896)[None, :] >= (np.arange(128)[:, None] + 384)
    ).astype(NPBF16)
    selg = np.zeros((64, 8 * 128), dtype=NPBF16)
    for r in range(8):
        for h in range(2):
            selg[32 * h + r, r * 128 + 64 * h : r * 128 + 64 * h + 64] = 1

    in_maps = []
    for c in range(NCORES):
        sl = slice(FEAT * c, FEAT * (c + 1))
        in_maps.append(
            {
                "xT": xT,
                "wq": np.ascontiguousarray(wq_full[:, sl]).astype(NPBF16),
                "wk": np.ascontiguousarray(wk_full[:, sl]).astype(NPBF16),
                "wv": np.ascontiguousarray(wv_full[:, sl]).astype(NPBF16),
                "bq": np.ascontiguousarray(bq_full[sl].reshape(FEAT, 1)),
                "bk": np.ascontiguousarray(bk_full[sl].reshape(FEAT, 1)),
                "bv": np.ascontiguousarray(bv_full[sl].reshape(FEAT, 1)),
                "wout": wout_b,
                "boutb": boutb,
                "maskg": maskg,
                "selg": selg,
            }
        )

    nc = _get_nc()
    res = run_bass_kernel_spmd(
        nc,
        in_maps,
        core_ids=list(range(NCORES)),
        trace=TRACE,
        **TRACE_KWARGS,
    )
    LAST_RESULT = res
    out = np.empty((B, T, D), dtype=np.float32)
    for c in range(NCORES):
        yc = res.results[c]["y"]
        out[0, c * TOKC : (c + 1) * TOKC, :] = yc[0:TOKC]
        out[1, c * TOKC : (c + 1) * TOKC, :] = yc[TOKC : 2 * TOKC]
    return out


# revision 17
# speedup vs baseline: 1.1699x; 1.1699x over previous
"""Trainium2 Bass kernel for nn_EngramAttention (causal MHA block).

Computes: qkv = x @ Wqkv + bqkv; causal 16-head attention; out @ Wout + bout.
Shapes: x [2, 2048, 1024], Wqkv [1024, 3072], Wout [1024, 1024].

Sharding (8 NeuronCores, tensor-parallel by heads):
  - core c owns heads {2c, 2c+1} (128 feature columns of each of Q/K/V).
  - Every core reads all tokens (x fed pre-transposed, feature-major, bf16).
  - Per batch b (= token half): QKV projection, causal attention in scoresT
    layout (keys on partitions; softmax denominator via an appended ones-row
    in the PV matmul).  Attention outputs are shipped UN-normalized together
    with the denominator row ([65, 256] blocks) through per-(batch, head)
    AllToAlls; each core ends with all 1024 features for its 256-token slice
    of each batch, normalizes post-A2A (one batched reciprocal + selection-
    matrix broadcast matmuls) and runs the output projection locally.
  - Emission weaves batch-1 QKV under batch-0 attention so the scalar engine
    (exp) starts early and the PE stream never starves (keeps HAM warm).

All matmuls run in bf16 (fp32 accumulation in PSUM).
"""

import os
import sys

for _p in ("/opt/trn_rl_repo", "/root/.axon_site/_ro/trn_rl_repo"):
    if os.path.isdir(_p) and _p not in sys.path:
        sys.path.insert(0, _p)

import ml_dtypes
import numpy as np

import concourse.bass as bass
import concourse.mybir as mybir
import concourse.tile as tile
from concourse.bass_utils import run_bass_kernel_spmd
from concourse.masks import make_identity
from concourse.vector_clock import ScopedClock

BF16 = mybir.dt.bfloat16
F32 = mybir.dt.float32
NPBF16 = ml_dtypes.bfloat16

NCORES = 8
D = 1024          # hidden
NTOK = 4096       # B*T
T = 2048
B = 2
FEAT = 128        # per-core head features (2 heads x 64)
TOKC = 256        # per-core per-batch token slice in the output projection
SCALE = 0.125     # 1/sqrt(64)

# module-level handles for optional tracing by test harnesses
TRACE = False
TRACE_KWARGS = {}
LAST_RESULT = None


class _SplitDrainTileContext(tile.TileContext):
    """TileContext whose tail drain splits semaphore waits one-per-instruction.

    The walrus build in this container rejects >N sync waits on a single
    Drain ("Too many sync wait commands"), so emit a chain of drains, each
    carrying a single wait, instead of one drain carrying all of them.
    """

    def _drain_and_barrier(self, tick_clock, wait_clock):
        nc = self.nc
        drain_inst = nc.sync.drain()
        wait_clock.add_sem_waits(
            drain_inst.ins, ScopedClock({None: tick_clock.global_clock})
        )
        si = drain_inst.ins.sync_info
        if si is not None and si.on_wait and len(si.on_wait) > 1:
            waits = list(si.on_wait)
            drain_inst.ins.sync_info = mybir.SyncInfo(
                on_wait=waits[:1], on_update=list(si.on_update or [])
            )
            for w in waits[1:]:
                d2 = nc.sync.drain()
                si2 = d2.ins.sync_info
                upd = list(si2.on_update or []) if si2 is not None else []
                d2.ins.sync_info = mybir.SyncInfo(on_wait=[w], on_update=upd)

        nc.all_engine_barrier()
        assert self.sems is not None
        popped = nc._tile_sem_poison_stack.pop()
        assert popped is self._sem_poison
        nc.clear_and_free_semaphores(list(self.sems.allocated().values()))
        nc.all_engine_barrier()


def _split_excess_waits(nc, aux, max_waits=1):
    """Walrus in this container rejects instructions carrying more than a
    couple of semaphore waits ("Too many sync wait commands").  Move excess
    waits onto EventSemaphore carrier instructions inserted just before the
    offending instruction on the same engine (same-engine FIFO order makes
    this semantically identical).

    DMA instructions execute on the DMA-queue processors, asynchronously
    from the issuing engine's stream, so an engine-side carrier alone would
    NOT order them (CoreSim race detector confirms).  For those, the carrier
    chain additionally increments an auxiliary semaphore and the DMA itself
    waits on it — the DMA then carries exactly one wait."""
    n = 0
    aux_count = 0
    dma_ops = ("DMACopy", "DMATranspose", "TriggeredCopy")

    def _carrier(engine, wait_grp):
        nonlocal n
        ev = mybir.InstEventSemaphore(
            name=f"wsplit-{n}",
            engine=engine,
            ins=[],
            outs=[],
            sync_info=mybir.SyncInfo(on_wait=list(wait_grp), on_update=[]),
        )
        n += 1
        nc.register_instruction(ev, overwrite=True)
        return ev

    for fn in nc.m.functions:
        for blk in fn.blocks:
            out = []
            for ins in blk.instructions:
                si = ins.sync_info
                waits = list(si.on_wait) if (si is not None and si.on_wait) else []
                if len(waits) > max_waits:
                    if ins.opcode in dma_ops:
                        for w in waits:
                            out.append(_carrier(ins.engine, [w]))
                        bass.BassInstruction(out[-1]).then_inc(aux, 1)
                        aux_count += 1
                        ins.sync_info = mybir.SyncInfo(
                            on_wait=[], on_update=list(si.on_update or [])
                        )
                        bass.BassInstruction(ins).wait_op(
                            aux, aux_count, "sem-ge"
                        )
                    else:
                        extra, keep = waits[:-max_waits], waits[-max_waits:]
                        for i in range(0, len(extra), max_waits):
                            out.append(_carrier(ins.engine, extra[i : i + max_waits]))
                        ins.sync_info = mybir.SyncInfo(
                            on_wait=keep, on_update=list(si.on_update or [])
                        )
                out.append(ins)
            blk.instructions = out
    if aux_count:
        # sems persist across NEFF executions; reset so a re-run starts at 0
        nc.gpsimd.sem_clear(range(aux.num, aux.num + 1))
    return n


def _weave(*streams, weights=None):
    """Emit thunks from several streams interleaved by fractional progress.
    A stream with weight w emits w times faster than a weight-1 stream, so
    higher-weight streams are front-loaded within the block."""
    pairs = [
        (list(s), (weights[i] if weights else 1.0))
        for i, s in enumerate(streams)
        if s
    ]
    idx = [0] * len(pairs)
    total = sum(len(s) for s, _ in pairs)
    for _ in range(total):
        k = min(
            range(len(pairs)),
            key=lambda i: (idx[i] / (len(pairs[i][0]) * pairs[i][1]), i)
            if idx[i] < len(pairs[i][0])
            else (9.0, i),
        )
        pairs[k][0][idx[k]]()
        idx[k] += 1


def _build_nc():
    nc = bass.Bass("TRN2", num_devices=NCORES)

    xT = nc.dram_tensor("xT", [D, NTOK], BF16, kind="ExternalInput")
    wq = nc.dram_tensor("wq", [D, FEAT], BF16, kind="ExternalInput")
    wk = nc.dram_tensor("wk", [D, FEAT], BF16, kind="ExternalInput")
    wv = nc.dram_tensor("wv", [D, FEAT], BF16, kind="ExternalInput")
    bq = nc.dram_tensor("bq", [FEAT, 1], F32, kind="ExternalInput")
    bk = nc.dram_tensor("bk", [FEAT, 1], F32, kind="ExternalInput")
    bv = nc.dram_tensor("bv", [FEAT, 1], F32, kind="ExternalInput")
    wout = nc.dram_tensor("wout", [D, D], BF16, kind="ExternalInput")
    boutb = nc.dram_tensor("boutb", [1, D], BF16, kind="ExternalInput")
    maskg = nc.dram_tensor("maskg", [128, 896], BF16, kind="ExternalInput")
    selg = nc.dram_tensor("selg", [64, 8 * 128], BF16, kind="ExternalInput")
    y = nc.dram_tensor("y", [2 * TOKC, D], F32, kind="ExternalOutput")

    # auxiliary semaphore for the DMA-wait splitting pass; allocated (and
    # cleared) before the TileContext so Tile never recycles its ID
    aux_sem = nc.alloc_semaphore("wsplit_aux")
    nc.gpsimd.sem_clear(range(aux_sem.num, aux_sem.num + 1))

    with _SplitDrainTileContext(nc) as tc:
        with (
            tc.tile_pool(name="const", bufs=1) as cp,
            tc.tile_pool(name="work", bufs=2) as wp,
            tc.tile_pool(name="vtp", bufs=2) as vp,
            tc.tile_pool(name="stage", bufs=2) as sp2,
            tc.tile_pool(name="psA", bufs=3, space="PSUM") as psA,
            tc.tile_pool(name="psB", bufs=2, space="PSUM") as psB,
            tc.tile_pool(name="dram", bufs=1, space="DRAM") as dp,
        ):
            # ---- persistent SBUF tensors ----
            xt_sb = cp.tile([128, 8 * NTOK], BF16, name="xt_sb")     # 64 KB/part
            wq_sb = cp.tile([128, 8 * FEAT], BF16, name="wq_sb")
            wk_sb = cp.tile([128, 8 * FEAT], BF16, name="wk_sb")
            wv_sb = cp.tile([128, 8 * FEAT], BF16, name="wv_sb")
            bq_sb = cp.tile([FEAT, 1], F32, name="bq_sb")
            bk_sb = cp.tile([FEAT, 1], F32, name="bk_sb")
            bv_sb = cp.tile([FEAT, 1], F32, name="bv_sb")
            bout_sb = cp.tile([1, D], BF16, name="bout_sb")
            mask_sb = cp.tile([128, 896], BF16, name="mask_sb")
            ident_sb = cp.tile([128, 128], BF16, name="ident_sb")
            ones1_sb = cp.tile([1, 128], BF16, name="ones1_sb")
            sel_sb = cp.tile([64, 8 * 128], BF16, name="sel_sb")
            dummy_sb = cp.tile([128, 128], BF16, name="dummy_sb")
            qT_sb = cp.tile([128, NTOK], BF16, name="qT_sb")
            kT_sb = cp.tile([128, NTOK], BF16, name="kT_sb")
            vtok_sb = cp.tile([128, 32 * 130], BF16, name="vtok_sb")
            wout_sb = cp.tile([128, 8 * D], BF16, name="wout_sb")
            ag_sb = [
                cp.tile([128, 8 * TOKC], BF16, name=f"ag_sb{b}") for b in range(2)
            ]
            denb_sb = [
                cp.tile([64, TOKC], BF16, name=f"denb_sb{b}") for b in range(2)
            ]
            denf_sb = [
                cp.tile([64, TOKC], F32, name=f"denf_sb{b}") for b in range(2)
            ]
            denr_sb = [
                cp.tile([64, TOKC], F32, name=f"denr_sb{b}") for b in range(2)
            ]
            denrb_sb = [
                cp.tile([64, TOKC], BF16, name=f"denrb_sb{b}") for b in range(2)
            ]

            # ---- input DMAs ----
            # xT (batch-0 half first) on the sync HWDGE ring; weights/bias/
            # mask go via the gpsimd SWDGE ring so they don't delay xT.
            for bb in range(2):
                for kt in range(8):
                    nc.sync.dma_start(
                        xt_sb[:, kt * NTOK + bb * T : kt * NTOK + (bb + 1) * T],
                        xT[kt * 128 : (kt + 1) * 128, bb * T : (bb + 1) * T],
                    )
            for w_sb, wdr in ((wq_sb, wq), (wk_sb, wk), (wv_sb, wv)):
                for kt in range(8):
                    nc.gpsimd.dma_start(
                        w_sb[:, kt * FEAT : (kt + 1) * FEAT],
                        wdr[kt * 128 : (kt + 1) * 128, :],
                    )
            nc.gpsimd.dma_start(bq_sb[:], bq[:])
            nc.gpsimd.dma_start(bk_sb[:], bk[:])
            nc.gpsimd.dma_start(bv_sb[:], bv[:])
            nc.gpsimd.dma_start(bout_sb[:], boutb[:])
            nc.gpsimd.dma_start(mask_sb[:], maskg[:])
            for kt in range(8):
                nc.gpsimd.dma_start(
                    wout_sb[:, kt * D : (kt + 1) * D],
                    wout[kt * 128 : (kt + 1) * 128, :],
                )

            make_identity(nc, ident_sb[:])
            nc.vector.memset(ones1_sb[:], 1.0)
            nc.vector.memset(dummy_sb[:], 1.0)
            vt_view = vtok_sb[:].rearrange("p (g c) -> p g c", c=130)
            nc.vector.memset(vt_view[:, :, 64], 1.0)
            nc.vector.memset(vt_view[:, :, 129], 1.0)
            # selection matrix (host-built): sel[32h+r, r*128 + 64h + i] = 1
            # (i < 64), so sel[:, r*128:(r+1)*128].T @ denrb broadcasts den
            # row 32h+r onto output partitions [64h, 64h+64) for block r.
            # (h=1 rows live at partition 32 — engine APs need 32-aligned
            # partition starts.)  Unused denrb rows must be zero so the
            # broadcast matmul never multiplies 0 * garbage.
            nc.gpsimd.dma_start(sel_sb[:], selg[:])
            for bb in range(2):
                nc.vector.memset(denrb_sb[bb][:], 0.0)

            _dummy_phase = [0]

            def dummy_ops(nmm, pool=None, tag="mm2"):
                """Full-utilization keep-warm matmuls with no data deps.
                Operands come from wout (random data) so the PE datapath
                actually toggles — constant operands don't register as
                activity for the clock-gate monitor."""
                ops = []
                group = 8
                for i in range(0, nmm, group):
                    def op(n=min(group, nmm - i), pool=pool or psA, tag=tag):
                        ps_d = pool.tile([128, 128], F32, tag=tag, name="ps_d")
                        for _ in range(n):
                            j = _dummy_phase[0] = (_dummy_phase[0] + 5) % 9
                            nc.tensor.matmul(
                                ps_d[:],
                                xt_sb[:, j * 128 : (j + 1) * 128],
                                xt_sb[:, (j + 7) * 128 : (j + 8) * 128],
                                start=True, stop=True,
                            )
                    ops.append(op)
                return ops

            # ---- QKV projection: dstT[f, tok] = W.T @ x.T (+ bias) ----
            def qkv_ops(which, bb, ts=(0, 1), box=None):
                w_sb, b_sb, dst = {
                    "q": (wq_sb, bq_sb, qT_sb),
                    "k": (wk_sb, bk_sb, kT_sb),
                    "v": (wv_sb, bv_sb, None),
                }[which]
                ops = []
                if box is None:
                    box = {}
                for t in ts:
                    def alloc(t=t):
                        if which == "v" and "vt" not in box:
                            box["vt"] = vp.tile([128, T], BF16, name=f"vT{bb}")
                        box["ps"] = psA.tile(
                            [128, 1024], F32, tag="mm2", name=f"ps_{which}{bb}{t}"
                        )
                    ops.append(alloc)
                    for kt in range(8):
                        def op(t=t, kt=kt):
                            ps = box["ps"]
                            for c in range(2):
                                base = bb * T + t * 1024 + c * 512
                                nc.tensor.matmul(
                                    ps[:, c * 512 : (c + 1) * 512],
                                    w_sb[:, kt * FEAT : (kt + 1) * FEAT],
                                    xt_sb[:, kt * NTOK + base : kt * NTOK + base + 512],
                                    start=(kt == 0),
                                    stop=(kt == 7),
                                )
                        ops.append(op)

                    def bias(t=t):
                        ps = box["ps"]
                        d = box["vt"] if which == "v" else dst
                        off = t * 1024 if which == "v" else bb * T + t * 1024
                        nc.vector.tensor_scalar_add(
                            d[:, off : off + 1024], ps[:], b_sb[:]
                        )
                    ops.append(bias)
                return ops, box

            # ---- v to token-major (PE transposes), with ones columns ----
            def vpost_ops(bb, vbox):
                ops = []
                for gl in range(16):
                    def op(gl=gl):
                        g = bb * 16 + gl
                        ps_t = psA.tile([128, 128], BF16, tag="mm2", name="ps_t")
                        nc.tensor.transpose(
                            ps_t[:], vbox["vt"][:, gl * 128 : (gl + 1) * 128],
                            ident_sb[:],
                        )
                        nc.vector.tensor_copy(
                            vtok_sb[:, g * 130 : g * 130 + 64], ps_t[:, 0:64]
                        )
                        nc.vector.tensor_copy(
                            vtok_sb[:, g * 130 + 65 : g * 130 + 129],
                            ps_t[:, 64:128],
                        )
                    ops.append(op)
                return ops

            # ---- attention stages ----
            # stage s = (h, b, j): q-chunk j (512 tokens of batch b), head
            # half h.  scoresT layout: [128 keys, 512 q] blocks, exp on ACT,
            # diagonal blocks masked, PV accumulates [65, 512] (row 64 =
            # softmax denominator via the vtok ones column).  The result is
            # shipped UN-normalized + denominator through the A2A.
            pt_tiles = {}
            a2a_in = {}
            a2a_out = {}
            for bb in range(2):
                for h in range(2):
                    a2a_in[bb, h] = dp.tile(
                        [8, 65, TOKC], BF16, name=f"a2a_in{bb}{h}"
                    )
                    a2a_out[bb, h] = dp.tile(
                        [8, 65, TOKC], BF16, name=f"a2a_out{bb}{h}"
                    )

            def scores_ops(s):
                h, bb, j = s
                nk = 4 * (j + 1)
                pt = wp.tile(
                    [128, nk * 512], BF16, tag="pt", name=f"pt_{h}_{bb}_{j}"
                )
                pt_tiles[s] = pt
                pb, cb = 64 * h, bb * T
                ops = []
                for kp in range(nk // 2):
                    def op(kp=kp, pt=pt, pb=pb, cb=cb, j=j):
                        ps2 = psA.tile([128, 1024], F32, tag="mm2", name="ps2")
                        for c in range(2):
                            kk = 2 * kp + c
                            nc.tensor.matmul(
                                ps2[:, c * 512 : (c + 1) * 512],
                                kT_sb[
                                    pb : pb + 64,
                                    cb + kk * 128 : cb + (kk + 1) * 128,
                                ],
                                qT_sb[
                                    pb : pb + 64,
                                    cb + j * 512 : cb + (j + 1) * 512,
                                ],
                                start=True,
                                stop=True,
                            )
                        nc.scalar.activation(
                            pt[:, (2 * kp) * 512 : (2 * kp + 2) * 512],
                            ps2[:],
                            mybir.ActivationFunctionType.Exp,
                            scale=SCALE,
                        )
                        for c in range(2):
                            kk = 2 * kp + c
                            if kk >= 4 * j:
                                i = kk - 4 * j
                                nc.vector.tensor_tensor(
                                    pt[:, kk * 512 : (kk + 1) * 512],
                                    pt[:, kk * 512 : (kk + 1) * 512],
                                    mask_sb[:, 384 - 128 * i : 896 - 128 * i],
                                    mybir.AluOpType.mult,
                                )
                    ops.append(op)
                return ops

            def pv_ops(s):
                h, bb, j = s
                nk = 4 * (j + 1)
                pt = pt_tiles.pop(s)
                ps_box = {}
                ops = []
                for kk in range(nk):
                    def op(kk=kk, pt=pt, h=h, bb=bb, nk=nk):
                        if kk == 0:
                            ps_box["o"] = psB.tile(
                                [65, 512], F32, tag="pv", name="ps_o"
                            )
                        g = bb * 16 + kk
                        nc.tensor.matmul(
                            ps_box["o"][:],
                            vtok_sb[:, g * 130 + 65 * h : g * 130 + 65 * h + 65],
                            pt[:, kk * 512 : (kk + 1) * 512],
                            start=(kk == 0),
                            stop=(kk == nk - 1),
                        )
                    ops.append(op)

                def ship(h=h, bb=bb, j=j):
                    ps_o = ps_box["o"]
                    av = sp2.tile([65, 512], BF16, tag="av", name="av")
                    nc.vector.tensor_copy(av[:], ps_o[:])
                    for t in range(2):
                        nc.sync.dma_start(
                            a2a_in[bb, h][2 * j + t],
                            av[:, t * TOKC : (t + 1) * TOKC],
                        )
                ops.append(ship)
                return ops

            def emit_collective(bb, h):
                nc.gpsimd.collective_compute(
                    "AllToAll",
                    mybir.AluOpType.bypass,
                    replica_groups=[list(range(NCORES))],
                    ins=[a2a_in[bb, h][:]],
                    outs=[a2a_out[bb, h][:]],
                )
                nc.gpsimd.dma_start(
                    denb_sb[bb][32 * h : 32 * h + 8, :],
                    a2a_out[bb, h][:, 64, :],
                )
                for r in range(8):
                    nc.gpsimd.dma_start(
                        ag_sb[bb][
                            64 * h : 64 * h + 64, r * TOKC : (r + 1) * TOKC
                        ],
                        a2a_out[bb, h][r, 0:64, :],
                    )

            # ---- post-A2A normalization + output projection ----
            def recip_ops(bb, h):
                def op():
                    sl = slice(32 * h, 32 * h + 8)
                    nc.vector.tensor_copy(denf_sb[bb][sl, :], denb_sb[bb][sl, :])
                    nc.vector.reciprocal(denr_sb[bb][sl, :], denf_sb[bb][sl, :])
                    nc.vector.tensor_copy(denrb_sb[bb][sl, :], denr_sb[bb][sl, :])
                return [op]

            def norm_ops(bb):
                ops = []
                for r in range(8):
                    def op(r=r):
                        ps_bc = psA.tile(
                            [128, TOKC], F32, tag="mm2", name="ps_bc"
                        )
                        nc.tensor.matmul(
                            ps_bc[:],
                            sel_sb[:, r * 128 : (r + 1) * 128],
                            denrb_sb[bb][:, :],
                            start=True,
                            stop=True,
                        )
                        nc.vector.tensor_tensor(
                            ag_sb[bb][:, r * TOKC : (r + 1) * TOKC],
                            ag_sb[bb][:, r * TOKC : (r + 1) * TOKC],
                            ps_bc[:],
                            mybir.AluOpType.mult,
                        )
                    ops.append(op)
                return ops

            def outproj_ops(bb):
                ops = []
                for m in range(2):
                    box = {}
                    def mm0(m=m, box=box):
                        box["ps"] = psA.tile(
                            [128, 1024], F32, tag="mm2", name=f"ps_y{bb}{m}"
                        )
                        for n2 in range(2):
                            nc.tensor.matmul(
                                box["ps"][:, n2 * 512 : (n2 + 1) * 512],
                                ones1_sb[0:1, 0:128],
                                bout_sb[:, n2 * 512 : (n2 + 1) * 512],
                                start=True,
                                stop=False,
                            )
                    ops.append(mm0)
                    for kt in range(8):
                        def op(m=m, kt=kt, box=box):
                            for n2 in range(2):
                                nc.tensor.matmul(
                                    box["ps"][:, n2 * 512 : (n2 + 1) * 512],
                                    ag_sb[bb][
                                        :,
                                        kt * TOKC + m * 128 :
                                        kt * TOKC + (m + 1) * 128,
                                    ],
                                    wout_sb[
                                        :,
                                        kt * D + n2 * 512 :
                                        kt * D + (n2 + 1) * 512,
                                    ],
                                    start=False,
                                    stop=(kt == 7),
                                )
                        ops.append(op)

                    def fin(m=m, box=box):
                        y_sb = sp2.tile([128, D], F32, tag="ysb", name="y_sb")
                        nc.vector.tensor_copy(y_sb[:], box["ps"][:])
                        nc.sync.dma_start(
                            y[bb * 256 + m * 128 : bb * 256 + (m + 1) * 128, :],
                            y_sb[:],
                        )
                    ops.append(fin)
                return ops

            def norm_half_ops(bb, h):
                """Normalize only the 64 feature rows of head-half h."""
                ops = []
                for r in range(8):
                    def op(r=r):
                        ps_bc = psB.tile(
                            [128, TOKC], F32, tag="pv", name="ps_bch"
                        )
                        nc.tensor.matmul(
                            ps_bc[64 * h : 64 * h + 64, :],
                            sel_sb[:, r * 128 + 64 * h : r * 128 + 64 * h + 64],
                            denrb_sb[bb][:, :],
                            start=True,
                            stop=True,
                        )
                        nc.vector.tensor_tensor(
                            ag_sb[bb][64 * h : 64 * h + 64,
                                      r * TOKC : (r + 1) * TOKC],
                            ag_sb[bb][64 * h : 64 * h + 64,
                                      r * TOKC : (r + 1) * TOKC],
                            ps_bc[64 * h : 64 * h + 64, :],
                            mybir.AluOpType.mult,
                        )
                    ops.append(op)
                return ops

            def outproj_partial_ops(bb, h, boxes, first, last):
                """Half-contraction output projection over feature rows of
                head-half h only (rows 64h:64h+64 of every kt block).  The
                PSUM accumulation group stays open between the two halves so
                the h=0 half can run while the last A2A is in flight."""
                ops = []
                for m in range(2):
                    box = boxes[m]
                    if first:
                        def mm0(m=m, box=box):
                            box["ps"] = psA.tile(
                                [128, 1024], F32, tag="mm2", name=f"ps_yp{m}"
                            )
                            for n2 in range(2):
                                nc.tensor.matmul(
                                    box["ps"][:, n2 * 512 : (n2 + 1) * 512],
                                    ones1_sb[0:1, 0:128],
                                    bout_sb[:, n2 * 512 : (n2 + 1) * 512],
                                    start=True,
                                    stop=False,
                                )
                        ops.append(mm0)
                    for kt in range(8):
                        def op(m=m, kt=kt, box=box):
                            for n2 in range(2):
                                nc.tensor.matmul(
                                    box["ps"][:, n2 * 512 : (n2 + 1) * 512],
                                    ag_sb[bb][
                                        64 * h : 64 * h + 64,
                                        kt * TOKC + m * 128 :
                                        kt * TOKC + (m + 1) * 128,
                                    ],
                                    wout_sb[
                                        64 * h : 64 * h + 64,
                                        kt * D + n2 * 512 :
                                        kt * D + (n2 + 1) * 512,
                                    ],
                                    start=False,
                                    stop=(last and kt == 7),
                                )
                        ops.append(op)
                    if last:
                        def fin(m=m, box=box):
                            y_sb = sp2.tile(
                                [128, D], F32, tag="ysb", name="y_sb"
                            )
                            nc.vector.tensor_copy(y_sb[:], box["ps"][:])
                            nc.sync.dma_start(
                                y[bb * 256 + m * 128 :
                                  bb * 256 + (m + 1) * 128, :],
                                y_sb[:],
                            )
                        ops.append(fin)
                return ops

            # ================= emission schedule =================
            # Phase A: only the first token-half of batch-0 K and Q — the
            # j=0,1 attention stages need nothing else, so the scalar engine
            # (exp, the long pole) starts ~25us earlier.  Warm-up dummies
            # keep the PE array active through the DMA-gated start.
            kops_t0, _ = qkv_ops("k", 0, ts=(0,))
            _weave(kops_t0, dummy_ops(40))
            for op in qkv_ops("q", 0, ts=(0,))[0]:
                op()

            # Rest of batch-0 QKV (k/q t1, v both halves) woven into the
            # first b0 stage; the v transposes are emitted as a block after
            # it so no b0 PV op can queue ahead of the transpose it needs.
            vbox0 = {}
            rest0 = (
                qkv_ops("k", 0, ts=(1,))[0]
                + qkv_ops("q", 0, ts=(1,))[0]
                + qkv_ops("v", 0, box=vbox0)[0]
            )
            rest0_chunks = [rest0] + [[]] * 7

            kops1, _ = qkv_ops("k", 1)
            qops1, _ = qkv_ops("q", 1)
            vops1, vbox1 = qkv_ops("v", 1)
            qkv1 = kops1 + qops1 + vops1 + vpost_ops(1, vbox1)
            nq = len(qkv1)
            # spread batch-1 QKV over b0 stages 1..7 (full-util matmuls keep
            # the PE activity monitor from throttling the clock)
            qkv1_chunks = [[]] + [
                qkv1[nq * i // 7 : nq * (i + 1) // 7] for i in range(7)
            ]

            # batch-0 post work woven into batch-1 stages 3..7; batch-1
            # h0 stages get dummy full-util matmuls as activity filler.
            post0 = (
                recip_ops(0, 0)
                + recip_ops(0, 1)
                + norm_ops(0)
                + outproj_ops(0)
            )
            np0 = len(post0)
            post0_chunks = [
                dummy_ops(16), dummy_ops(16), dummy_ops(16), dummy_ops(16)
            ] + [
                post0[np0 * i // 4 : np0 * (i + 1) // 4] for i in range(4)
            ]

            stages = [(h, bb, j) for bb in range(2) for h in range(2)
                      for j in range(4)]
            prev = None
            for si, s in enumerate(stages):
                h, bb, j = s
                a = scores_ops(s)
                b = pv_ops(prev) if prev is not None else []
                if bb == 0:
                    if si == 0:
                        # first stage: scores first so exp starts ASAP, the
                        # remaining QKV work follows behind it
                        for op in a:
                            op()
                        _weave(rest0_chunks[si], qkv1_chunks[si])
                    else:
                        _weave(a, b, rest0_chunks[si], qkv1_chunks[si])
                else:
                    _weave(a, b, post0_chunks[si - 8])
                if si == 0:
                    for op in vpost_ops(0, vbox0):
                        op()
                if prev is not None and prev[2] == 3:
                    emit_collective(prev[1], prev[0])
                prev = s
            for op in pv_ops(prev):
                op()
            emit_collective(1, 1)

            # tail: h0 half of the batch-1 normalization + output projection
            # runs while the last A2A (which carries the h1 half) flies; the
            # PSUM accumulation groups stay open across the boundary.
            yboxes = [{}, {}]
            for op in recip_ops(1, 0) + norm_half_ops(1, 0):
                op()
            for op in outproj_partial_ops(1, 0, yboxes, first=True,
                                          last=False):
                op()
            for op in dummy_ops(48, pool=psB, tag="pv"):
                op()
            for op in recip_ops(1, 1) + norm_half_ops(1, 1):
                op()
            for op in outproj_partial_ops(1, 1, yboxes, first=False,
                                          last=True):
                op()

    _split_excess_waits(nc, aux_sem)
    return nc


_NC_CACHE = None


def _get_nc():
    global _NC_CACHE
    if _NC_CACHE is None:
        _NC_CACHE = _build_nc()
    return _NC_CACHE


def kernel(x, Wqkv, bqkv, Wout, bout):
    global LAST_RESULT
    x = np.asarray(x, dtype=np.float32)
    Wqkv = np.asarray(Wqkv, dtype=np.float32)
    bqkv = np.asarray(bqkv, dtype=np.float32)
    Wout = np.asarray(Wout, dtype=np.float32)
    bout = np.asarray(bout, dtype=np.float32)

    Bx, Tx, Dx = x.shape
    assert (Bx, Tx, Dx) == (B, T, D)

    xT = np.ascontiguousarray(x.reshape(NTOK, D).T).astype(NPBF16)
    wq_full = Wqkv[:, 0:D]
    wk_full = Wqkv[:, D : 2 * D]
    wv_full = Wqkv[:, 2 * D : 3 * D]
    bq_full = bqkv[0:D]
    bk_full = bqkv[D : 2 * D]
    bv_full = bqkv[2 * D : 3 * D]

    wout_b = np.ascontiguousarray(Wout).astype(NPBF16)
    boutb = np.ascontiguousarray(bout.reshape(1, D)).astype(NPBF16)
    maskg = (
        np.arange(896)[None, :] >= (np.arange(128)[:, None] + 384)
    ).astype(NPBF16)
    selg = np.zeros((64, 8 * 128), dtype=NPBF16)
    for r in range(8):
        for h in range(2):
            selg[32 * h + r, r * 128 + 64 * h : r * 128 + 64 * h + 64] = 1

    in_maps = []
    for c in range(NCORES):
        sl = slice(FEAT * c, FEAT * (c + 1))
        in_maps.append(
            {
                "xT": xT,
                "wq": np.ascontiguousarray(wq_full[:, sl]).astype(NPBF16),
                "wk": np.ascontiguousarray(wk_full[:, sl]).astype(NPBF16),
                "wv": np.ascontiguousarray(wv_full[:, sl]).astype(NPBF16),
                "bq": np.ascontiguousarray(bq_full[sl].reshape(FEAT, 1)),
                "bk": np.ascontiguousarray(bk_full[sl].reshape(FEAT, 1)),
                "bv": np.ascontiguousarray(bv_full[sl].reshape(FEAT, 1)),
                "wout": wout_b,
                "boutb": boutb,
                "maskg": maskg,
                "selg": selg,
            }
        )

    nc = _get_nc()
    res = run_bass_kernel_spmd(
        nc,
        in_maps,
        core_ids=list(range(NCORES)),
        trace=TRACE,
        **TRACE_KWARGS,
    )
    LAST_RESULT = res
    out = np.empty((B, T, D), dtype=np.float32)
    for c in range(NCORES):
        yc = res.results[c]["y"]
        out[0, c * TOKC : (c + 1) * TOKC, :] = yc[0:TOKC]
        out[1, c * TOKC : (c + 1) * TOKC, :] = yc[TOKC : 2 * TOKC]
    return out
